# revision 55
# baseline (speedup 1.0000x reference)
"""Trainium2 8-core kernel for a single-head AttentionBlock.

Reference computation (fp32, per batch b):
    qkv = x @ w_qkv.T + b_qkv            # [S, 3H]
    q, k, v = split(qkv)                 # each [S, H]
    scores = q @ k.T / sqrt(H)           # [S, S]
    probs = softmax(scores, -1)
    ctx = probs @ v                      # [S, H]
    out = ctx @ w_out.T + b_out          # [S, H]

Shapes: B=4, S=2048, H=2048 (single head, head_dim = H).

Sharding: 8 cores = 4 batches x 2 query-halves. Core c handles batch
b = c // 2 and queries qc*1024 ... (qc+1)*1024 with qc = c % 2. Each core
projects K/V for its own sequence half; the two cores of a batch exchange
K/V halves with pairwise AllGathers so each has the full K/V for attention.

Compute is bf16 on the TensorEngine with fp32 PSUM accumulation; softmax
runs in fp32 (exp on ScalarE). The 1/sqrt(H) scale and all layout
transposes are folded on the host. Measured rel err vs the fp32 reference
is ~5e-3.

Scheduling (from iterating on hardware traces; 767us baseline -> ~685us,
PE-cycle floor for this sharding is ~660us):
 - All four collectives are 2MB halves (K by feature-chunk lo/hi, V by
   output-feature lo/hi) so the serial CC stream starts at ~95us and is
   fully drained by ~305us -- the whole attention phase runs with zero
   collective bandwidth contention.  AG(k_lo) fires mid-k-proj.
 - First x s-block arrives as 4-ht-group quarters and the first six
   k-proj output groups consume only that block (defer_s1), hiding the
   second s-block's 2MB DMA; the first four k weight slabs are
   prefetched ahead of it in the Sync FIFO.
 - Scores' first-half K slabs (skg 0/1, qb=0) and the first ctx v-slab
   live in const-pool tiles prefetched on the gpsimd queue right after
   their AGs complete; pool tiles can't prefetch early because phase-A
   pool allocs WAR-wait on ALL phase-P tile deaths.
 - v-proj staging casts on the ScalarE (no bias add: b_v folds exactly
   into the out-proj bias as b_out + w_out @ b_v since it passes
   additively through the softmax-normalized ctx).  Keeping the DVE out
   of that path removes a store->DVE->PSUM-release stall chain.
 - V is stored partition-major ([p, st, o]) so one ctx DMA fetches 4
   key chunks -- the Sync sequencer's ~0.6us per-DMA issue cost was
   pacing the ctx phase at 128 separate loads.
 - Softmax denominator: ones[128,128] @ den broadcast-reduce matmul
   (injected after the first 4 ctx accumulation steps, den pre-cast to
   bf16) + full-width [128,512] reciprocal on DVE.  Replaces a [1,512]
   reciprocal (3.4us on one partition) + two broadcast matmuls that
   stalled the PE; norms are interleaved hg-wise so exactly 8 PSUM
   banks are ever live.
"""

import math
import os

import numpy as np
import ml_dtypes

import concourse.bacc as bacc
import concourse.tile as tile
import concourse.mybir as mybir
from concourse.bass_utils import run_bass_kernel_spmd
from concourse.tile import add_dep_helper

BF16 = ml_dtypes.bfloat16
F32 = mybir.dt.float32
BF = mybir.dt.bfloat16

B, S, H = 4, 2048, 2048
SQ = S // 2          # queries per core
HT = H // 128        # 16 h-chunks
N_CORES = 8

USE_COLLECTIVE = os.environ.get("ATTN_USE_COLLECTIVE", "1") == "1"

REPLICA_GROUPS = [[0, 1], [2, 3], [4, 5], [6, 7]]


def build_graph(use_collective: bool = USE_COLLECTIVE):
    nc = bacc.Bacc(
        "TRN2", target_bir_lowering=False, debug=False, num_devices=N_CORES
    )

    # ---- DRAM parameters (per-core shards, host-prepared layouts) ----
    # xt[p, sb, ht, s] = x_local[sb*512+s, ht*128+p]   (batch row,
    # transposed, s-block-major so each x-tile DMA reads 4-16KB contiguous
    # runs per partition — 1KB elements throttled the startup loads;
    # collective mode only ships the core's own sequence half)
    S_LOC = SQ if use_collective else S
    NSB = S_LOC // 512
    xt_e = nc.dram_tensor("xt", [128, NSB, HT, 512], BF, kind="ExternalInput")
    # wqk[ot, p, ht, m] = w_qkv[ot*128+m, ht*128+p]; ot 0..15 = q (pre-scaled
    # by 1/sqrt(H)), ot 16..31 = k.
    wqk_e = nc.dram_tensor("wqk", [32, 128, HT, 128], BF, kind="ExternalInput")
    # wv[ob, p, ht, n] = w_qkv[2H + ob*512+n, ht*128+p]
    wv_e = nc.dram_tensor("wv", [4, 128, HT, 512], BF, kind="ExternalInput")
    # wo[ob, p, ht, n] = w_out[ob*512+n, ht*128+p]
    wo_e = nc.dram_tensor("wo", [4, 128, HT, 512], BF, kind="ExternalInput")
    # bqk[p, t]: t 0..15 q bias (pre-scaled), 16..31 k bias
    bqk_e = nc.dram_tensor("bqk", [128, 32], F32, kind="ExternalInput")
    # combined output bias (b_out + wo @ b_v), broadcast along partitions
    # (the v bias passes additively through the softmax-normalized ctx, so
    # it folds exactly into the out-proj bias on the host)
    bob_e = nc.dram_tensor("bob", [128, H], BF, kind="ExternalInput")

    out_e = nc.dram_tensor("out", [SQ, H], F32, kind="ExternalOutput")

    # ---- internal DRAM: K^T and V, stored per sequence-half ----
    # kt[half][p, kc, s_in_half] = k[half*1024+s, kc*128+p], split into
    # lo/hi k-feature-chunk halves so AG(k_lo) can start mid-k-proj and
    # the serial CC stream finishes all three collectives earlier.
    # v[half][p, st, o] = v[half*1024 + st*128 + p, o]  (partition-major so
    # one ctx DMA can fetch 4 consecutive 128-row key chunks — the Sync
    # sequencer's ~0.6us per-DMA issue cost was pacing the ctx phase)
    if use_collective:
        kt_sh_lo = nc.dram_tensor("kt_sh_lo", [128, HT // 2, SQ], BF)
        kt_sh_hi = nc.dram_tensor("kt_sh_hi", [128, HT // 2, SQ], BF)
        v_sh_lo = nc.dram_tensor("v_sh_lo", [128, SQ // 128, H // 2], BF)
        v_sh_hi = nc.dram_tensor("v_sh_hi", [128, SQ // 128, H // 2], BF)
        # NB: Shared-output collectives need >4-core groups; pairs use Local.
        kt_d_lo = nc.dram_tensor("kt_g_lo", [2, 128, HT // 2, SQ], BF)
        kt_d_hi = nc.dram_tensor("kt_g_hi", [2, 128, HT // 2, SQ], BF)
        v_d_lo = nc.dram_tensor("v_g_lo", [2, 128, SQ // 128, H // 2], BF)
        v_d_hi = nc.dram_tensor("v_g_hi", [2, 128, SQ // 128, H // 2], BF)
    else:
        kt_d_lo = nc.dram_tensor("kt_d_lo", [2, 128, HT // 2, SQ], BF)
        kt_d_hi = nc.dram_tensor("kt_d_hi", [2, 128, HT // 2, SQ], BF)
        v_d_lo = nc.dram_tensor("v_d_lo", [2, 128, SQ // 128, H // 2], BF)
        v_d_hi = nc.dram_tensor("v_d_hi", [2, 128, SQ // 128, H // 2], BF)

    HQ = 4               # ht chunks per x tile (DMA granularity)
    NHG = HT // HQ       # 4 ht-groups

    with tile.TileContext(nc) as tc:
        with (
            tc.tile_pool(name="const", bufs=1) as cpool,
            tc.tile_pool(name="small", bufs=1) as spool,
            tc.tile_pool(name="psum", bufs=8, space="PSUM") as pp,
        ):
            # persistent tiles
            qT = cpool.tile([128, HT, SQ], BF, tag="qT")       # 32KB/p
            ctxT = cpool.tile([128, HT, SQ], BF, tag="ctxT")   # 32KB/p
            # first-half K slabs (skg 0/1), prefetched on the gpsimd queue
            # right after AG(k) so scores never wait on the Sync DMA FIFO
            # (phase-A pool allocs wait on ALL of phase P via the pool
            # stack, so a pooled tile can't be prefetched early)
            ks_preA = cpool.tile([128, HT, 512], BF, tag="ks_preA")  # 16KB/p
            ks_preB = cpool.tile([128, HT, 512], BF, tag="ks_preB")  # 16KB/p
            # first ctx v-slab, same prefetch trick (the vs pool's first
            # load is WAR-blocked until the phase-P pools die at q-proj end)
            vs_pre = cpool.tile([128, 4, 512], BF, tag="vs_pre")     # 4KB/p
            bqk = cpool.tile([128, 32], F32, tag="bqk")
            bob = cpool.tile([128, H], BF, tag="bob")
            ones128 = cpool.tile([128, 128], BF, tag="ones128")

            nc.vector.memset(ones128[:], 1.0)

            # ================= Phase P: projections =================
            with (
                tc.tile_pool(name="xt", bufs=1) as xpool,
                tc.tile_pool(name="wqk", bufs=6) as wqkp,
                tc.tile_pool(name="wv", bufs=2) as wvp,
                tc.tile_pool(name="stg", bufs=2) as stg,
                tc.tile_pool(name="vstg", bufs=8) as vstg,
            ):
                # x tiles: the first 512-wide s-block arrives as 4-ht-group
                # quarters so the first matmul group only waits for 512KB;
                # the later s-blocks are single whole-tile DMAs.
                NSB = S_LOC // 512
                xt0q = [
                    xpool.tile([128, HQ, 512], BF, tag=f"xt0_{hg}",
                               name=f"xts0_{hg}")
                    for hg in range(NHG)
                ]
                xts = [
                    xpool.tile([128, HT, 512], BF, tag=f"xt{sb}",
                               name=f"xts{sb}")
                    for sb in range(1, NSB)
                ]
                # first weight slabs ahead of the xt bulk so the early
                # matmul groups' stationary operands aren't queued behind
                # the 2MB second x s-block
                w_pre = {}
                if use_collective:
                    for ot in range(16, 20):
                        w = wqkp.tile([128, HT, 128], BF, tag="wqk",
                                      name=f"w_pre{ot}")
                        if ot > 16:
                            # first quarter of x goes between slab 16 and
                            # the rest so the very first group can start
                            nc.sync.dma_start(
                                out=xt0q[ot - 17][:],
                                in_=xt_e[:, 0, (ot - 17) * HQ:(ot - 16) * HQ, :],
                            )
                        nc.sync.dma_start(out=w[:], in_=wqk_e[ot])
                        w_pre[ot] = w
                    nc.sync.dma_start(
                        out=xt0q[3][:], in_=xt_e[:, 0, 3 * HQ:4 * HQ, :]
                    )
                else:
                    for hg in range(NHG):
                        nc.sync.dma_start(
                            out=xt0q[hg][:],
                            in_=xt_e[:, 0, hg * HQ:(hg + 1) * HQ, :],
                        )
                for sb in range(1, NSB):
                    nc.sync.dma_start(
                        out=xts[sb - 1][:],
                        in_=xt_e[:, sb],
                    )
                # bqk is a 128B-per-partition DMA whose descriptor issue
                # alone costs ~5us on the Sync sequencer — park it on the
                # idle gpsimd queue instead (needed only at ~25us)
                nc.gpsimd.dma_start(out=bqk[:], in_=bqk_e.ap())

                def xt_sl(ht, s0, width):
                    sb, off = divmod(s0, 512)
                    assert off + width <= 512
                    if sb == 0:
                        hg, hh = divmod(ht, HQ)
                        return xt0q[hg][:, hh, off:off + width]
                    return xts[sb - 1][:, ht, off:off + width]

                # Log projection weight DMAs so AG-dependent Sync DMAs can
                # be order-pinned after them (the Sync DMA stream is a
                # FIFO: a DMA that waits on an AllGather would
                # head-of-line-block everything scheduled after it).
                w_dma_log = []

                def proj_qk(ot_list, s_lo, s_hi, is_q, pre=None,
                            defer_s1=0):
                    # produces output-transposed tiles [o(128 part), s]
                    # defer_s1: that many leading ot groups run their
                    # s0=0 block first and their s0=512 block only after
                    # all of them — covers the second x s-block's DMA at
                    # kernel start without idling the PE.
                    w_tiles = {}

                    def get_w(idx, ot):
                        if ot not in w_tiles:
                            if pre is not None and ot in pre:
                                w_tiles[ot] = pre[ot]
                            else:
                                w = wqkp.tile([128, HT, 128], BF, tag="wqk")
                                w_dma_log.append(
                                    nc.sync.dma_start(out=w[:], in_=wqk_e[ot])
                                )
                                w_tiles[ot] = w
                        return w_tiles[ot]

                    order = []
                    lead = list(ot_list)[:defer_s1]
                    for idx, ot in enumerate(lead):
                        order.append((idx, ot, s_lo, s_lo + 512))
                    for idx, ot in enumerate(lead):
                        order.append((idx, ot, s_lo + 512, s_hi))
                    for idx, ot in enumerate(ot_list):
                        if idx < defer_s1:
                            continue
                        order.append((idx, ot, s_lo, s_hi))
                    for idx, ot, lo, hi in order:
                        w = get_w(idx, ot)
                        for s0 in range(lo, hi, 512):
                            ps = pp.tile([128, 512], F32, tag="ps")
                            for ht in range(HT):
                                nc.tensor.matmul(
                                    ps[:],
                                    w[:, ht, :],
                                    xt_sl(ht, s0, 512),
                                    start=(ht == 0),
                                    stop=(ht == HT - 1),
                                )
                            if is_q:
                                nc.scalar.activation(
                                    qT[:, ot, s0:s0 + 512],
                                    ps[:],
                                    mybir.ActivationFunctionType.Identity,
                                    bias=bqk[:, ot:ot + 1],
                                )
                            else:
                                kst = stg.tile([128, 512], BF, tag="kst")
                                nc.scalar.activation(
                                    kst[:],
                                    ps[:],
                                    mybir.ActivationFunctionType.Identity,
                                    bias=bqk[:, ot:ot + 1],
                                )
                                half, off = divmod(s0, SQ)
                                kc = ot - 16
                                if use_collective:
                                    dst = kt_sh_lo if kc < 8 else kt_sh_hi
                                else:
                                    dst = (kt_d_lo if kc < 8 else kt_d_hi)[half]
                                nc.sync.dma_start(
                                    out=dst[:, kc % 8, off:off + 512],
                                    in_=kst[:],
                                )

                def proj_v(st_list):
                    for ob in range(4):
                        w = wvp.tile([128, HT, 512], BF, tag="wv")
                        w_dma_log.append(
                            nc.sync.dma_start(out=w[:], in_=wv_e[ob])
                        )
                        for st in st_list:
                            ps = pp.tile([128, 512], F32, tag="ps")
                            for ht in range(HT):
                                nc.tensor.matmul(
                                    ps[:],
                                    xt_sl(ht, st * 128, 128),
                                    w[:, ht, :],
                                    start=(ht == 0),
                                    stop=(ht == HT - 1),
                                )
                            # cast on the scalar engine: keeps the DVE (and
                            # its queue's DMA-completion waits) off the
                            # v-proj PSUM-release path entirely
                            vst = vstg.tile([128, 512], BF, tag="vst")
                            nc.scalar.activation(
                                vst[:],
                                ps[:],
                                mybir.ActivationFunctionType.Identity,
                            )
                            half, sti = divmod(st, SQ // 128)
                            if use_collective:
                                dst = v_sh_lo if ob < 2 else v_sh_hi
                            else:
                                dst = (v_d_lo if ob < 2 else v_d_hi)[half]
                            nc.sync.dma_start(
                                out=dst[:, sti,
                                        (ob % 2) * 512:(ob % 2 + 1) * 512],
                                in_=vst[:],
                            )

                # SPMD: the graph is identical on all cores, so each core
                # projects K/V for the FIRST half of its LOCAL sequence
                # order. The host ships each core its own query-half, so a
                # core's K/V shard is exactly its own half; the pairwise
                # AllGather produces [2, ...] buffers in GLOBAL half order
                # for both cores of a batch. Attention just sums over both
                # halves, so key order never affects the result.
                def prefetch_ks_pre():
                    # prefetch both half-0 score k-slabs on the idle gpsimd
                    # queue (waits on AG(k) there without blocking anything)
                    nc.gpsimd.dma_start(
                        out=ks_preA[:, 0:8, :], in_=kt_d_lo[0][:, :, 0:512]
                    )
                    nc.gpsimd.dma_start(
                        out=ks_preA[:, 8:16, :], in_=kt_d_hi[0][:, :, 0:512]
                    )
                    nc.gpsimd.dma_start(
                        out=ks_preB[:, 0:8, :], in_=kt_d_lo[0][:, :, 512:1024]
                    )
                    nc.gpsimd.dma_start(
                        out=ks_preB[:, 8:16, :], in_=kt_d_hi[0][:, :, 512:1024]
                    )

                if use_collective:
                    proj_qk(range(16, 24), 0, SQ, False, pre=w_pre,
                            defer_s1=6)                              # k lo
                    nc.gpsimd.collective_compute(
                        "AllGather",
                        mybir.AluOpType.bypass,
                        replica_groups=REPLICA_GROUPS,
                        ins=[kt_sh_lo.ap().opt()],
                        outs=[kt_d_lo.ap().opt()],
                    )
                    proj_qk(range(24, 32), 0, SQ, False)             # k hi
                    nc.gpsimd.collective_compute(
                        "AllGather",
                        mybir.AluOpType.bypass,
                        replica_groups=REPLICA_GROUPS,
                        ins=[kt_sh_hi.ap().opt()],
                        outs=[kt_d_hi.ap().opt()],
                    )
                    proj_v(range(SQ // 128))              # v own half
                    nc.gpsimd.collective_compute(
                        "AllGather",
                        mybir.AluOpType.bypass,
                        replica_groups=REPLICA_GROUPS,
                        ins=[v_sh_lo.ap().opt()],
                        outs=[v_d_lo.ap().opt()],
                    )
                    nc.gpsimd.collective_compute(
                        "AllGather",
                        mybir.AluOpType.bypass,
                        replica_groups=REPLICA_GROUPS,
                        ins=[v_sh_hi.ap().opt()],
                        outs=[v_d_hi.ap().opt()],
                    )
                    prefetch_ks_pre()
                    nc.gpsimd.dma_start(
                        out=vs_pre[:], in_=v_d_lo[0][:, 0:4, 0:512]
                    )
                    proj_qk(range(16), 0, SQ, True)       # q (own half)
                else:
                    proj_qk(range(16), 0, SQ, True)
                    proj_qk(range(16, 32), 0, S, False)
                    proj_v(range(S // 128))
                    prefetch_ks_pre()
                    nc.gpsimd.dma_start(
                        out=vs_pre[:], in_=v_d_lo[0][:, 0:4, 0:512]
                    )

            # ================= Phase A: attention + out proj =========
            with (
                tc.tile_pool(name="ks", bufs=2) as kp,
                tc.tile_pool(name="probs", bufs=1) as prp,
                tc.tile_pool(name="vs", bufs=3) as vp,
                tc.tile_pool(name="wo", bufs=2) as wop,
                tc.tile_pool(name="ost", bufs=3) as op,
            ):
                # pin AG-dependent DMAs after the weight stream so they
                # can't head-of-line-block it in the Sync FIFO
                vs_gate = w_dma_log[-1] if w_dma_log else None

                def gated_dma(out, in_, gate):
                    d = nc.sync.dma_start(out=out, in_=in_)
                    if gate is not None:
                        add_dep_helper(
                            d.ins,
                            gate.ins,
                            sync=False,
                            reason="AG-dependent DMA after weight stream",
                        )
                    return d

                nc.sync.dma_start(out=bob[:], in_=bob_e.ap())
                for qb in range(2):
                    q_sl = slice(qb * 512, (qb + 1) * 512)
                    probs = prp.tile([128, 16, 512], BF, tag="probs")
                    den = spool.tile([128, 512], F32, tag="den")
                    # ---- scores + exp.  qb=0's first two k-slabs come
                    # from the gpsimd-prefetched const tiles (the Sync
                    # FIFO can't deliver them in time during the AG(v)
                    # window); by qb=1 the collectives are long done, so
                    # the normal pool path is fine. ----
                    for skg in range(4):
                        half, off = divmod(skg * 512, SQ)
                        if qb == 0 and skg == 0:
                            ks = ks_preA
                        elif qb == 0 and skg == 1:
                            ks = ks_preB
                        else:
                            ks = kp.tile([128, HT, 512], BF, tag="ks")
                            gated_dma(
                                ks[:, 0:8, :],
                                kt_d_lo[half][:, :, off:off + 512],
                                vs_gate,
                            )
                            gated_dma(
                                ks[:, 8:16, :],
                                kt_d_hi[half][:, :, off:off + 512],
                                vs_gate,
                            )
                        for skw in range(4):
                            sk = skg * 4 + skw
                            ps = pp.tile([128, 512], F32, tag="ps")
                            for ht in range(HT):
                                nc.tensor.matmul(
                                    ps[:],
                                    ks[:, ht, skw * 128:(skw + 1) * 128],
                                    qT[:, ht, q_sl],
                                    start=(ht == 0),
                                    stop=(ht == HT - 1),
                                )
                            nc.scalar.activation(
                                probs[:, sk, :],
                                ps[:],
                                mybir.ActivationFunctionType.Exp,
                            )
                            if sk == 0:
                                nc.vector.tensor_copy(den[:], probs[:, 0, :])
                            else:
                                nc.vector.tensor_add(
                                    den[:], den[:], probs[:, sk, :]
                                )
                    rb = spool.tile([128, 512], F32, tag="rb")

                    # ---- ctx^T accumulation, h-groups of 4.  After the
                    # first 4 accumulation steps of hg 0 (once the DVE den
                    # chain has surely drained), reduce+broadcast den with
                    # a single ones[128,128] matmul and take the
                    # reciprocal at full width on DVE; both finish well
                    # before hg 1 so only 8 PSUM banks are ever live. ----
                    def ctx_hg(hg, inject=None):
                        cps = [
                            pp.tile([128, 512], F32, tag="ps", name=f"cps{i}")
                            for i in range(4)
                        ]
                        v_half = v_d_lo if hg < 2 else v_d_hi
                        for skq in range(4):
                            half, stq = divmod(skq * 4, SQ // 128)
                            if qb == 0 and hg == 0 and skq == 0:
                                vs = vs_pre
                            else:
                                vs = vp.tile([128, 4, 512], BF, tag="vs")
                                gated_dma(
                                    vs[:],
                                    v_half[half][
                                        :, stq:stq + 4,
                                        (hg % 2) * 512:(hg % 2 + 1) * 512],
                                    vs_gate,
                                )
                            for j in range(4):
                                sk = skq * 4 + j
                                for hl in range(4):
                                    nc.tensor.matmul(
                                        cps[hl][:],
                                        vs[:, j, hl * 128:(hl + 1) * 128],
                                        probs[:, sk, :],
                                        start=(sk == 0),
                                        stop=(sk == 15),
                                    )
                            if skq == 0 and inject is not None:
                                inject()
                        return cps

                    def den_reduce():
                        den_bf = spool.tile([128, 512], BF, tag="den_bf")
                        nc.vector.tensor_copy(den_bf[:], den[:])
                        dbc = pp.tile([128, 512], F32, tag="ps", name="dbc")
                        nc.tensor.matmul(
                            dbc[:], ones128[:], den_bf[:], start=True, stop=True
                        )
                        nc.vector.reciprocal(rb[:], dbc[:])

                    def ctx_norm(hg, cps):
                        for hl in range(4):
                            nc.vector.tensor_mul(
                                ctxT[:, hg * 4 + hl, q_sl], cps[hl][:], rb[:]
                            )

                    cps0 = ctx_hg(0, inject=den_reduce)
                    cps1 = ctx_hg(1)
                    ctx_norm(0, cps0)
                    cps2 = ctx_hg(2)
                    ctx_norm(1, cps1)
                    cps3 = ctx_hg(3)
                    ctx_norm(2, cps2)
                    ctx_norm(3, cps3)
                # ---- output projection ----
                for ob in range(4):
                    w = wop.tile([128, HT, 512], BF, tag="wo")
                    nc.sync.dma_start(out=w[:], in_=wo_e[ob])
                    for st in range(SQ // 128):
                        ps = pp.tile([128, 512], F32, tag="ps")
                        for ht in range(HT):
                            nc.tensor.matmul(
                                ps[:],
                                ctxT[:, ht, st * 128:(st + 1) * 128],
                                w[:, ht, :],
                                start=(ht == 0),
                                stop=(ht == HT - 1),
                            )
                        ost = op.tile([128, 512], F32, tag="ost")
                        nc.vector.tensor_add(
                            ost[:], ps[:], bob[:, ob * 512:(ob + 1) * 512]
                        )
                        nc.sync.dma_start(
                            out=out_e[st * 128:(st + 1) * 128,
                                      ob * 512:(ob + 1) * 512],
                            in_=ost[:],
                        )

    nc.compile()
    return nc


def prep_inputs(hidden_states, w_qkv, b_qkv, w_out, b_out, use_collective=USE_COLLECTIVE):
    """Build the 8 per-core input maps (host-side sharding + layout)."""
    hidden_states = np.asarray(hidden_states, dtype=np.float32)
    w_qkv = np.asarray(w_qkv, dtype=np.float32)
    b_qkv = np.asarray(b_qkv, dtype=np.float32)
    w_out = np.asarray(w_out, dtype=np.float32)
    b_out = np.asarray(b_out, dtype=np.float32)

    scale = 1.0 / math.sqrt(H)
    wq = w_qkv[:H] * scale
    wk = w_qkv[H: 2 * H]
    wv = w_qkv[2 * H:]

    # wqk[ot, p, ht, m] = w[ot*128+m, ht*128+p]
    wqk_parts = np.concatenate([wq, wk], axis=0)  # [2H, H]
    wqk_l = np.ascontiguousarray(
        wqk_parts.reshape(32, 128, HT, 128).transpose(0, 3, 2, 1)
    ).astype(BF16)
    # wv[ob, p, ht, n] = wv[ob*512+n, ht*128+p]
    wv_l = np.ascontiguousarray(
        wv.reshape(4, 512, HT, 128).transpose(0, 3, 2, 1)
    ).astype(BF16)
    wo_l = np.ascontiguousarray(
        w_out.reshape(4, 512, HT, 128).transpose(0, 3, 2, 1)
    ).astype(BF16)

    bq = b_qkv[:H] * scale
    bk = b_qkv[H: 2 * H]
    bqk_l = np.ascontiguousarray(
        np.concatenate([bq, bk]).reshape(32, 128).T
    ).astype(np.float32)
    # v-bias folded into the out-proj bias: (ctx + b_v) @ w_out.T + b_out
    # == ctx @ w_out.T + (b_out + w_out @ b_v)
    b_comb = b_out + w_out @ b_qkv[2 * H:]
    bob_l = np.ascontiguousarray(np.broadcast_to(b_comb, (128, H))).astype(
        BF16
    )

    in_maps = []
    for core in range(N_CORES):
        b, qc = divmod(core, 2)
        x = hidden_states[b]  # [S, H]
        if use_collective:
            x = x[qc * SQ:(qc + 1) * SQ]  # own half only
        elif qc == 1:
            # local sequence order: own half first
            x = np.concatenate([x[SQ:], x[:SQ]], axis=0)
        s_loc = x.shape[0]
        # xt[p, sb, ht, s] = x[sb*512+s, ht*128+p]
        nsb = s_loc // 512
        xt = np.ascontiguousarray(
            x.T.reshape(HT, 128, nsb, 512).transpose(1, 2, 0, 3)
        ).astype(BF16)
        in_maps.append(
            {
                "xt": xt,
                "wqk": wqk_l,
                "wv": wv_l,
                "wo": wo_l,
                "bqk": bqk_l,
                "bob": bob_l,
            }
        )
    return in_maps


_CACHED = {}


def _get_graph(use_collective=USE_COLLECTIVE):
    key = bool(use_collective)
    if key not in _CACHED:
        _CACHED[key] = build_graph(key)
    return _CACHED[key]


def run(hidden_states, w_qkv, b_qkv, w_out, b_out, trace=False,
        use_collective=USE_COLLECTIVE):
    nc = _get_graph(use_collective)
    in_maps = prep_inputs(
        hidden_states, w_qkv, b_qkv, w_out, b_out, use_collective
    )
    res = run_bass_kernel_spmd(
        nc, in_maps, list(range(N_CORES)), trace=trace
    )
    out = np.empty((B, S, H), dtype=np.float32)
    for core in range(N_CORES):
        b, qc = divmod(core, 2)
        out[b, qc * SQ:(qc + 1) * SQ] = res.results[core]["out"]
    return out, res


def kernel(hidden_states, w_qkv, b_qkv, w_out, b_out):
    out, _ = run(hidden_states, w_qkv, b_qkv, w_out, b_out)
    return out


if __name__ == "__main__":
    rng = np.random.default_rng(0)
    hs = rng.standard_normal((B, S, H)).astype(np.float32)
    a1 = math.sqrt(6.0 / (H + 3 * H))
    a2 = math.sqrt(6.0 / (2 * H))
    wq = rng.uniform(-a1, a1, (3 * H, H)).astype(np.float32)
    wo = rng.uniform(-a2, a2, (H, H)).astype(np.float32)
    out = kernel(hs, wq, np.zeros(3 * H, np.float32), wo, np.zeros(H, np.float32))
    print(out.shape, out.dtype)
